# revision 3
# baseline (speedup 1.0000x reference)
"""Trainium2 8-core Bass kernel for a dual cross-attention transformer block.

Sharding: tensor-parallel attention heads (16 heads -> 2/core) for both the
x-side and y-side attention; token-parallel FFN (4096 tokens -> 512/core) with
full FFN weights replicated.  Comms: AllGather of the RMS-normed, transposed
activations (fp8) at the start; AllToAll of the per-head attention outputs
(fp8) per side, after which each core applies the FULL Wo to its own 512-token
shard locally (no ReduceScatter).  All activations live feature-on-partition
("transposed") so biases and norm weights are per-partition scalars.  Compute
dtype fp8 (DoubleRow) for all GEMMs except attention scores (bf16); fp32 PSUM.
"""

import math

import numpy as np
import ml_dtypes

import concourse.bass as bass
import concourse.tile as tile
from concourse import mybir, bacc
from concourse.bass_utils import run_bass_kernel_spmd

B, S, D, H = 2, 2048, 2048, 16
HD = D // H            # 128
HID = 5632
EPS = 1e-5
NC = 8                 # cores
HPC = H // NC          # 2 heads per core
T = B * S              # 4096 tokens
TPC = T // NC          # 512 tokens per core
KCH = D // 128         # 16 contraction chunks over D
JCH = HID // 128       # 44 chunks over HID
QBLK = 512             # token block for QKV projections
NQB = T // QBLK        # 8
BF = ml_dtypes.bfloat16
E4 = ml_dtypes.float8_e4m3
F32 = mybir.dt.float32
BF16 = mybir.dt.bfloat16
FP8 = mybir.dt.float8e4
import os as _os
USE_FP8 = _os.environ.get("BASS_FP8", "1") == "1"
USE_FP8_QKV = _os.environ.get("BASS_FP8_QKV", "1") == "1"
USE_A2A = _os.environ.get("BASS_A2A", "1") == "1"
FSC = 16.0
OSC = 16.0             # attention-output fp8 scale (A2A payload)

_CACHE = {}


def _rope_perm():
    # [evens, odds]: puts x1 in partitions 0:64, x2 in 64:128 of Q^T/K^T
    return np.concatenate([np.arange(0, 128, 2), np.arange(1, 128, 2)])


def build_nc(sim_local=False, reps=1, fp8=None):
    if fp8 is None:
        fp8 = USE_FP8
    fp8_qkv = fp8 and USE_FP8_QKV
    use_a2a = USE_A2A and fp8
    nc = bacc.Bacc("TRN2", target_bir_lowering=False, debug=False,
                   num_devices=1 if sim_local else NC)

    AL = mybir.AluOpType
    AF = mybir.ActivationFunctionType

    # ---------------- external parameters ----------------
    xT = {s: nc.declare_dram_parameter(f"{s}T", [D, TPC], F32, isOutput=False)
          for s in ("x", "y")}
    # rope tables, stacked [c;c] and [-s;s]; q-variant pre-scaled by 1/sqrt(HD)
    tabs = {n: nc.declare_dram_parameter(n, [128, S], F32, isOutput=False)
            for n in ("cs_q", "sn_q", "cs_k", "sn_k")}
    wq, wk, wv, wo = {}, {}, {}, {}
    bq, bqs, bk, bks, bv, bo = {}, {}, {}, {}, {}, {}
    w1, w3, w2, b1, b3, b2, fnw = {}, {}, {}, {}, {}, {}, {}
    for s in ("x", "y"):
        qdt = FP8 if fp8_qkv else BF16
        wq[s] = nc.declare_dram_parameter(f"wq_{s}", [HPC, 128, KCH, 128], qdt, isOutput=False)
        wk[s] = nc.declare_dram_parameter(f"wk_{s}", [HPC, 128, KCH, 128], qdt, isOutput=False)
        wv[s] = nc.declare_dram_parameter(f"wv_{s}", [128, KCH, HPC * 128], qdt, isOutput=False)
        if use_a2a:
            # full Wo, fp8, tiled per output chunk for DoubleRow over head pairs
            wo[s] = nc.declare_dram_parameter(f"wo_{s}", [KCH, 128, H // 2, 2, 128],
                                              FP8, isOutput=False)
        else:
            wo[s] = nc.declare_dram_parameter(f"wo_{s}", [HPC, 128, KCH, 128], BF16, isOutput=False)
        bq[s] = nc.declare_dram_parameter(f"bq_{s}", [HPC, 128], F32, isOutput=False)
        bqs[s] = nc.declare_dram_parameter(f"bqs_{s}", [HPC, 128], F32, isOutput=False)
        bk[s] = nc.declare_dram_parameter(f"bk_{s}", [HPC, 128], F32, isOutput=False)
        bks[s] = nc.declare_dram_parameter(f"bks_{s}", [HPC, 128], F32, isOutput=False)
        bv[s] = nc.declare_dram_parameter(f"bv_{s}", [HPC * 128], F32, isOutput=False)
        bo[s] = nc.declare_dram_parameter(f"bo_{s}", [KCH, 128], F32, isOutput=False)
        wdt = FP8 if fp8 else BF16
        w1[s] = nc.declare_dram_parameter(f"w1_{s}", [JCH, 128, KCH, 128], wdt, isOutput=False)
        w3[s] = nc.declare_dram_parameter(f"w3_{s}", [JCH, 128, KCH, 128], wdt, isOutput=False)
        w2[s] = nc.declare_dram_parameter(f"w2_{s}", [KCH, 128, JCH, 128], wdt, isOutput=False)
        b1[s] = nc.declare_dram_parameter(f"b1_{s}", [JCH, 128], F32, isOutput=False)
        b3[s] = nc.declare_dram_parameter(f"b3_{s}", [JCH, 128], F32, isOutput=False)
        b2[s] = nc.declare_dram_parameter(f"b2_{s}", [KCH, 128], F32, isOutput=False)
        fnw[s] = nc.declare_dram_parameter(f"fnw_{s}", [KCH, 128], F32, isOutput=False)
    anw = nc.declare_dram_parameter("anw", [KCH, 128], F32, isOutput=False)
    out_ext = nc.declare_dram_parameter("out", [2, D, TPC], F32, isOutput=True)

    # ---------------- internal DRAM ----------------
    adt = FP8 if fp8_qkv else BF16
    ag_in = {(s, h): nc.dram_tensor(f"ag_in_{s}{h}", [D, TPC // 2], adt)
             for s in ("x", "y") for h in (0, 1)}
    ag_out = {(s, h): nc.dram_tensor(f"ag_out_{s}{h}", [NC, D, TPC // 2], adt,
                                     addr_space="Shared")
              for s in ("x", "y") for h in (0, 1)}
    v_dram = {s: nc.dram_tensor(f"v_{s}", [T, HPC * 128], BF16) for s in ("x", "y")}
    if use_a2a:
        a2a_in = {s: nc.dram_tensor(f"a2a_in_{s}", [NC, HPC * 128, TPC], FP8)
                  for s in ("x", "y")}
        a2a_out = {s: nc.dram_tensor(f"a2a_out_{s}", [NC, HPC * 128, TPC], FP8)
                   for s in ("x", "y")}
    else:
        rs_in = {s: nc.dram_tensor(f"rs_in_{s}", [NC, D, TPC], BF16) for s in ("x", "y")}
        rs_out = {s: nc.dram_tensor(f"rs_out_{s}", [D, TPC], BF16)
                  for s in ("x", "y")}

    rg = [list(range(NC))]

    from contextlib import ExitStack
    with tile.TileContext(nc) as tc:
        with ExitStack() as es:
            const = es.enter_context(tc.tile_pool(name="const", bufs=1))
            ones_bf = const.tile([128, 1], BF16)
            nc.vector.memset(ones_bf, 1.0)
            ones_f = const.tile([128, 1], F32)
            nc.vector.memset(ones_f, 1.0)
            ones_row = const.tile([1, 128], F32)
            nc.vector.memset(ones_row, 1.0)
            sc_row = const.tile([1, 128], F32)
            nc.vector.memset(sc_row, FSC if fp8_qkv else 1.0)
            eps_sb = const.tile([128, 1], F32)
            nc.vector.memset(eps_sb, EPS)
            anw_sb = const.tile([128, KCH], F32)
            nc.sync.dma_start(out=anw_sb, in_=anw.rearrange("k p -> p k"))
            fnw_sb, bo_sb, b2_sb = {}, {}, {}
            bq_sb, bqs_sb, bk_sb, bks_sb, bv_sb = {}, {}, {}, {}, {}
            b1_sb, b3_sb = {}, {}
            for s in ("x", "y"):
                fnw_sb[s] = const.tile([128, KCH], F32, name=f"fnw{s}", tag=f"fnw{s}")
                nc.sync.dma_start(out=fnw_sb[s], in_=fnw[s].rearrange("k p -> p k"))
                bo_sb[s] = const.tile([128, KCH], F32, name=f"bo{s}", tag=f"bo{s}")
                nc.sync.dma_start(out=bo_sb[s], in_=bo[s].rearrange("k p -> p k"))
                b2_sb[s] = const.tile([128, KCH], F32, name=f"b2{s}", tag=f"b2{s}")
                nc.sync.dma_start(out=b2_sb[s], in_=b2[s].rearrange("k p -> p k"))
                b1_sb[s] = const.tile([128, JCH], F32, name=f"b1{s}", tag=f"b1{s}")
                nc.sync.dma_start(out=b1_sb[s], in_=b1[s].rearrange("k p -> p k"))
                b3_sb[s] = const.tile([128, JCH], F32, name=f"b3{s}", tag=f"b3{s}")
                nc.sync.dma_start(out=b3_sb[s], in_=b3[s].rearrange("k p -> p k"))
                bq_sb[s] = const.tile([128, HPC], F32, name=f"bq{s}", tag=f"bq{s}")
                nc.sync.dma_start(out=bq_sb[s], in_=bq[s].rearrange("h p -> p h"))
                bqs_sb[s] = const.tile([128, HPC], F32, name=f"bqs{s}", tag=f"bqs{s}")
                nc.sync.dma_start(out=bqs_sb[s], in_=bqs[s].rearrange("h p -> p h"))
                bk_sb[s] = const.tile([128, HPC], F32, name=f"bk{s}", tag=f"bk{s}")
                nc.sync.dma_start(out=bk_sb[s], in_=bk[s].rearrange("h p -> p h"))
                bks_sb[s] = const.tile([128, HPC], F32, name=f"bks{s}", tag=f"bks{s}")
                nc.sync.dma_start(out=bks_sb[s], in_=bks[s].rearrange("h p -> p h"))
                bv_sb[s] = const.tile([128, HPC * 128], F32, name=f"bv{s}", tag=f"bv{s}")
                nc.sync.dma_start(out=bv_sb[s],
                                  in_=bv[s][None, :].to_broadcast([128, HPC * 128]))

            for _rep in range(reps):
                # ---------- phase 1: rms-norm own token shard, write ag_in ----------
                with tc.tile_pool(name="nrm", bufs=2) as nrm, \
                     tc.tile_pool(name="nrm_ps", bufs=2, space="PSUM") as nrm_ps:
                    for s in ("x", "y"):
                        xt_sb = []
                        ms_ps = nrm_ps.tile([1, TPC], F32, name="ms", tag="ms")
                        for kc in range(KCH):
                            t = nrm.tile([128, TPC], F32, name="xt", tag="xt", bufs=18)
                            nc.sync.dma_start(out=t, in_=xT[s][kc * 128:(kc + 1) * 128, :])
                            xt_sb.append(t)
                            sq = nrm.tile([128, TPC], F32, name="sq", tag="sq")
                            nc.scalar.activation(out=sq, in_=t, func=AF.Square)
                            nc.tensor.matmul(ms_ps[:, 0:TPC], ones_f, sq,
                                             start=(kc == 0), stop=(kc == KCH - 1))
                        sd = nrm.tile([1, TPC], F32, name="sd", tag="sd")
                        nc.scalar.activation(out=sd, in_=ms_ps, func=AF.Sqrt,
                                             bias=eps_sb[0:1, :], scale=1.0 / D)
                        rec = nrm.tile([1, TPC], F32, name="rec", tag="rec")
                        nc.vector.reciprocal(out=rec, in_=sd)
                        rb_ps = nrm_ps.tile([128, TPC], F32, name="rb", tag="rb")
                        nc.tensor.matmul(rb_ps, sc_row, rec, start=True, stop=True)
                        rb = nrm.tile([128, TPC], F32, name="rbs", tag="rbs")
                        nc.scalar.copy(out=rb, in_=rb_ps)
                        for kc in range(KCH):
                            nt = nrm.tile([128, TPC], FP8 if fp8_qkv else BF16,
                                          name="nt", tag="nt")
                            nc.vector.scalar_tensor_tensor(
                                out=nt, in0=xt_sb[kc], scalar=anw_sb[:, kc:kc + 1],
                                in1=rb, op0=AL.mult, op1=AL.mult)
                            for h in (0, 1):
                                nc.sync.dma_start(
                                    out=ag_in[(s, h)][kc * 128:(kc + 1) * 128, :],
                                    in_=nt[:, h * (TPC // 2):(h + 1) * (TPC // 2)])

                # ---------- phase 2: all-gather normalized activations ----------
                for h in (0, 1):
                    for s in ("x", "y"):
                        if sim_local:
                            for g in range(NC):
                                nc.sync.dma_start(out=ag_out[(s, h)][g],
                                                  in_=ag_in[(s, h)][:])
                        else:
                            nc.gpsimd.collective_compute(
                                "AllGather", AL.bypass, replica_groups=rg,
                                ins=[ag_in[(s, h)][:]], outs=[ag_out[(s, h)][:]])

                # ---------- phase 3: QKV projections (both sides) ----------
                qkt_es = ExitStack()
                qkt = qkt_es.enter_context(tc.tile_pool(name="qkt", bufs=1))
                qt_sb, kt_sb = {}, {}
                for s in ("x", "y"):
                    for h in range(HPC):
                        qt_sb[(s, h)] = qkt.tile([128, T], BF16, name=f"qt{s}{h}", tag=f"qt{s}{h}")
                        kt_sb[(s, h)] = qkt.tile([128, T], BF16, name=f"kt{s}{h}", tag=f"kt{s}{h}")

                with tc.tile_pool(name="qkvw", bufs=1) as qkvw:
                    wq_sb, wk_sb, wv_sb = {}, {}, {}
                    for s in ("x", "y"):
                        for h in range(HPC):
                            wq_sb[(s, h)] = qkvw.tile([128, KCH, 128], FP8 if fp8_qkv else BF16, name=f"wq{s}{h}", tag=f"wq{s}{h}")
                            nc.sync.dma_start(out=wq_sb[(s, h)], in_=wq[s][h])
                            wk_sb[(s, h)] = qkvw.tile([128, KCH, 128], FP8 if fp8_qkv else BF16, name=f"wk{s}{h}", tag=f"wk{s}{h}")
                            nc.sync.dma_start(out=wk_sb[(s, h)], in_=wk[s][h])
                        wv_sb[s] = qkvw.tile([128, KCH, HPC * 128], FP8 if fp8_qkv else BF16, name=f"wv{s}", tag=f"wv{s}")
                        nc.sync.dma_start(out=wv_sb[s], in_=wv[s][:])

                    with tc.tile_pool(name="acts", bufs=2) as acts, \
                         tc.tile_pool(name="ropet", bufs=2) as ropet, \
                         tc.tile_pool(name="ropes", bufs=3) as ropes, \
                         tc.tile_pool(name="qkv_ps", bufs=2, space="PSUM") as qkv_ps:
                        for qb in range(NQB):
                            pos = (qb * QBLK) % S
                            a_sb = {}
                            for s in ("x", "y"):
                                a = acts.tile([128, KCH, QBLK], FP8 if fp8_qkv else BF16, name=f"a{s}", tag=f"a{s}")
                                for half in (0, 1):
                                    nc.sync.dma_start(
                                        out=a[:, :, half * (TPC // 2):(half + 1) * (TPC // 2)],
                                        in_=ag_out[(s, half)][qb].rearrange(
                                            "(k p) t -> p k t", p=128))
                                a_sb[s] = a
                            tb = {}
                            for n in ("cs_q", "sn_q", "cs_k", "sn_k"):
                                tt = ropet.tile([128, QBLK], F32, tag=n)
                                nc.sync.dma_start(out=tt, in_=tabs[n][:, pos:pos + QBLK])
                                tb[n] = tt
                            for s in ("x", "y"):
                                src_q = a_sb["y" if s == "x" else "x"]  # queries from hidden
                                src_kv = a_sb[s]                        # keys/values from ctx
                                for h in range(HPC):
                                    for proj, wsb, bsb, bssb, cs_t, sn_t, dst in (
                                        ("q", wq_sb[(s, h)], bq_sb[s], bqs_sb[s],
                                         tb["cs_q"], tb["sn_q"], qt_sb[(s, h)]),
                                        ("k", wk_sb[(s, h)], bk_sb[s], bks_sb[s],
                                         tb["cs_k"], tb["sn_k"], kt_sb[(s, h)]),
                                    ):
                                        src = src_q if proj == "q" else src_kv
                                        ps = qkv_ps.tile([128, QBLK], F32, name="qk", tag="qk")
                                        if fp8_qkv:
                                            for kp in range(KCH // 2):
                                                nc.tensor.matmul(
                                                    ps, wsb[:, 2 * kp:2 * kp + 2, :],
                                                    src[:, 2 * kp:2 * kp + 2, :],
                                                    start=(kp == 0),
                                                    stop=(kp == KCH // 2 - 1),
                                                    perf_mode=mybir.MatmulPerfMode.DoubleRow)
                                        else:
                                            for kc in range(KCH):
                                                nc.tensor.matmul(
                                                    ps, wsb[:, kc, :], src[:, kc, :],
                                                    start=(kc == 0), stop=(kc == KCH - 1))
                                        # rope: copy PSUM->SBUF, swap halves via
                                        # DMA, then (q+b)*cs + (qsw+bsw)*sn
                                        qs = ropes.tile([128, QBLK], F32, name="qs", tag="qs")
                                        nc.scalar.copy(out=qs, in_=ps)
                                        qsw = ropes.tile([128, QBLK], F32, name="qsw", tag="qsw")
                                        nc.sync.dma_start(out=qsw[0:64, :], in_=qs[64:128, :])
                                        nc.sync.dma_start(out=qsw[64:128, :], in_=qs[0:64, :])
                                        t1 = ropes.tile([128, QBLK], F32, name="t1", tag="t1")
                                        nc.vector.scalar_tensor_tensor(
                                            out=t1, in0=qs, scalar=bsb[:, h:h + 1],
                                            in1=cs_t, op0=AL.add, op1=AL.mult)
                                        t2 = ropes.tile([128, QBLK], F32, name="t2", tag="t2")
                                        nc.vector.scalar_tensor_tensor(
                                            out=t2, in0=qsw, scalar=bssb[:, h:h + 1],
                                            in1=sn_t, op0=AL.add, op1=AL.mult)
                                        nc.vector.tensor_add(
                                            dst[:, qb * QBLK:(qb + 1) * QBLK], t1, t2)
                                # V in natural [token, hd] layout
                                for tk in range(QBLK // 128):
                                    vps = qkv_ps.tile([128, HPC * 128], F32, name="v", tag="v")
                                    if fp8_qkv:
                                        for kp in range(KCH // 2):
                                            nc.tensor.matmul(
                                                vps,
                                                src_kv[:, 2 * kp:2 * kp + 2,
                                                       tk * 128:(tk + 1) * 128],
                                                wv_sb[s][:, 2 * kp:2 * kp + 2, :],
                                                start=(kp == 0),
                                                stop=(kp == KCH // 2 - 1),
                                                perf_mode=mybir.MatmulPerfMode.DoubleRow)
                                    else:
                                        for kc in range(KCH):
                                            nc.tensor.matmul(
                                                vps, src_kv[:, kc, tk * 128:(tk + 1) * 128],
                                                wv_sb[s][:, kc, :],
                                                start=(kc == 0), stop=(kc == KCH - 1))
                                    vsb = ropes.tile([128, HPC * 128], BF16, name="vsb", tag="vsb")
                                    if fp8_qkv:
                                        # psum is (FSC*FSC)x scaled; rescale + bias
                                        nc.vector.scalar_tensor_tensor(
                                            out=vsb, in0=vps, scalar=1.0 / (FSC * FSC),
                                            in1=bv_sb[s], op0=AL.mult, op1=AL.add)
                                    else:
                                        nc.vector.tensor_add(vsb, vps, bv_sb[s])
                                    nc.sync.dma_start(
                                        out=v_dram[s][qb * QBLK + tk * 128:
                                                      qb * QBLK + (tk + 1) * 128, :],
                                        in_=vsb)

                # ---------- phase 4: attention per side; A2A of head outputs ----------
                for s in ("x", "y"):
                    if use_a2a:
                        with tc.tile_pool(name="att", bufs=2) as att, \
                             tc.tile_pool(name="att_pt", bufs=2) as att_pt, \
                             tc.tile_pool(name="att_ps", bufs=2, space="PSUM") as att_ps, \
                             tc.tile_pool(name="att_ps1", bufs=1, space="PSUM") as att_ps1:
                            for b in range(B):
                                for h in range(HPC):
                                    vsl = att.tile([128, KCH, 128], BF16, name="vsl", tag="vsl")
                                    nc.sync.dma_start(
                                        out=vsl,
                                        in_=v_dram[s][b * S:(b + 1) * S,
                                                      h * 128:(h + 1) * 128].rearrange(
                                            "(k p) d -> p k d", p=128))
                                    for blk in range(S // 1024):
                                        tq0 = b * S + blk * 1024
                                        pt = []
                                        for tkc in range(16):
                                            sps = att_ps.tile([128, 1024], F32, name="s", tag="s")
                                            for i in (0, 1):
                                                nc.tensor.matmul(
                                                    sps[:, i * 512:(i + 1) * 512],
                                                    kt_sb[(s, h)][:, b * S + tkc * 128:
                                                                  b * S + (tkc + 1) * 128],
                                                    qt_sb[(s, h)][:, tq0 + i * 512:
                                                                  tq0 + (i + 1) * 512],
                                                    start=True, stop=True)
                                            p = att_pt.tile([128, 1024], BF16, name="pt", tag="pt",
                                                            bufs=20)
                                            nc.scalar.activation(out=p, in_=sps, func=AF.Exp)
                                            pt.append(p)
                                        # denominator: DVE pair-tree over tk chunks,
                                        # then one ones-matmul for the partition sum
                                        lvl = pt
                                        li = 0
                                        while len(lvl) > 1:
                                            nxt = []
                                            for i in range(0, len(lvl), 2):
                                                dsum = att_pt.tile(
                                                    [128, 1024], BF16,
                                                    name="dsum", tag=f"ds{li}", bufs=3)
                                                nc.vector.tensor_add(dsum, lvl[i], lvl[i + 1])
                                                nxt.append(dsum)
                                            lvl = nxt
                                            li += 1
                                        ops = att_ps1.tile([128, 1024], F32, name="ops", tag="ops")
                                        den = att_ps1.tile([1, 1024], F32, name="den", tag="aux")
                                        for i in (0, 1):
                                            nc.tensor.matmul(
                                                den[:, i * 512:(i + 1) * 512], ones_bf,
                                                lvl[0][:, i * 512:(i + 1) * 512],
                                                start=True, stop=True)
                                        for tkc in range(16):
                                            for i in (0, 1):
                                                nc.tensor.matmul(
                                                    ops[:, i * 512:(i + 1) * 512],
                                                    vsl[:, tkc, :],
                                                    pt[tkc][:, i * 512:(i + 1) * 512],
                                                    start=(tkc == 0), stop=(tkc == 15))
                                        rec = att.tile([1, 1024], F32, name="rec", tag="rec")
                                        nc.vector.reciprocal(out=rec, in_=den)
                                        rb = att_ps1.tile([128, 1024], F32, name="rb", tag="aux")
                                        for i in (0, 1):
                                            nc.tensor.matmul(rb[:, i * 512:(i + 1) * 512],
                                                             ones_row,
                                                             rec[:, i * 512:(i + 1) * 512],
                                                             start=True, stop=True)
                                        rbs = att.tile([128, 1024], F32, name="rbs",
                                                       tag="rbs")
                                        nc.scalar.copy(out=rbs, in_=rb)
                                        po = att.tile([128, 1024], FP8, name="po",
                                                      tag="po", bufs=3)
                                        nc.vector.scalar_tensor_tensor(
                                            out=po, in0=ops, scalar=OSC,
                                            in1=rbs, op0=AL.mult, op1=AL.mult)
                                        g0 = tq0 // TPC
                                        for gg in (0, 1):
                                            nc.sync.dma_start(
                                                out=a2a_in[s][g0 + gg,
                                                              h * 128:(h + 1) * 128, :],
                                                in_=po[:, gg * 512:(gg + 1) * 512])
                        if sim_local:
                            for g in range(NC):
                                nc.sync.dma_start(out=a2a_out[s][g], in_=a2a_in[s][g])
                        else:
                            nc.gpsimd.collective_compute(
                                "AllToAll", AL.bypass, replica_groups=rg,
                                ins=[a2a_in[s][:]], outs=[a2a_out[s][:]])
                    else:
                        oT = {}
                        with tc.tile_pool(name=f"oT{s}", bufs=1) as oT_pool:
                            for h in range(HPC):
                                oT[h] = oT_pool.tile([128, T], BF16, name=f"o{h}", tag=f"o{h}")
                            with tc.tile_pool(name="att", bufs=2) as att, \
                                 tc.tile_pool(name="att_pt", bufs=2) as att_pt, \
                                 tc.tile_pool(name="att_ps", bufs=2, space="PSUM") as att_ps, \
                                 tc.tile_pool(name="att_ps1", bufs=1, space="PSUM") as att_ps1:
                                for b in range(B):
                                    for h in range(HPC):
                                        vsl = att.tile([128, KCH, 128], BF16, name="vsl", tag="vsl")
                                        nc.sync.dma_start(
                                            out=vsl,
                                            in_=v_dram[s][b * S:(b + 1) * S,
                                                          h * 128:(h + 1) * 128].rearrange(
                                                "(k p) d -> p k d", p=128))
                                        for blk in range(S // 1024):
                                            tq0 = b * S + blk * 1024
                                            pt = []
                                            for tkc in range(16):
                                                sps = att_ps.tile([128, 1024], F32, name="s", tag="s")
                                                for i in (0, 1):
                                                    nc.tensor.matmul(
                                                        sps[:, i * 512:(i + 1) * 512],
                                                        kt_sb[(s, h)][:, b * S + tkc * 128:
                                                                      b * S + (tkc + 1) * 128],
                                                        qt_sb[(s, h)][:, tq0 + i * 512:
                                                                      tq0 + (i + 1) * 512],
                                                        start=True, stop=True)
                                                p = att_pt.tile([128, 1024], BF16, name="pt", tag="pt",
                                                                bufs=20)
                                                nc.scalar.activation(out=p, in_=sps, func=AF.Exp)
                                                pt.append(p)
                                            lvl = pt
                                            li = 0
                                            while len(lvl) > 1:
                                                nxt = []
                                                for i in range(0, len(lvl), 2):
                                                    dsum = att_pt.tile(
                                                        [128, 1024], BF16,
                                                        name="dsum", tag=f"ds{li}", bufs=3)
                                                    nc.vector.tensor_add(dsum, lvl[i], lvl[i + 1])
                                                    nxt.append(dsum)
                                                lvl = nxt
                                                li += 1
                                            ops = att_ps1.tile([128, 1024], F32, name="ops", tag="ops")
                                            den = att_ps1.tile([1, 1024], F32, name="den", tag="aux")
                                            for i in (0, 1):
                                                nc.tensor.matmul(
                                                    den[:, i * 512:(i + 1) * 512], ones_bf,
                                                    lvl[0][:, i * 512:(i + 1) * 512],
                                                    start=True, stop=True)
                                            for tkc in range(16):
                                                for i in (0, 1):
                                                    nc.tensor.matmul(
                                                        ops[:, i * 512:(i + 1) * 512],
                                                        vsl[:, tkc, :],
                                                        pt[tkc][:, i * 512:(i + 1) * 512],
                                                        start=(tkc == 0), stop=(tkc == 15))
                                            rec = att.tile([1, 1024], F32, name="rec", tag="rec")
                                            nc.vector.reciprocal(out=rec, in_=den)
                                            rb = att_ps1.tile([128, 1024], F32, name="rb", tag="aux")
                                            for i in (0, 1):
                                                nc.tensor.matmul(rb[:, i * 512:(i + 1) * 512],
                                                                 ones_row,
                                                                 rec[:, i * 512:(i + 1) * 512],
                                                                 start=True, stop=True)
                                            rbs = att.tile([128, 1024], F32, name="rbs",
                                                           tag="rbs")
                                            nc.scalar.copy(out=rbs, in_=rb)
                                            nc.vector.tensor_mul(
                                                oT[h][:, tq0:tq0 + 1024], ops, rbs)

                            # Wo -> rs_in (token groups are the 512-token core shards)
                            with tc.tile_pool(name="wo", bufs=1) as wo_pool, \
                                 tc.tile_pool(name="wo_s", bufs=3) as wo_s, \
                                 tc.tile_pool(name="wo_ps", bufs=2, space="PSUM") as wo_ps:
                                wo_sb = {}
                                for h in range(HPC):
                                    wo_sb[h] = wo_pool.tile([128, KCH, 128], BF16, name=f"wo{h}", tag=f"wo{h}")
                                    nc.sync.dma_start(out=wo_sb[h], in_=wo[s][h])
                                for g2 in range(NC // 2):
                                    for kc in range(KCH):
                                        ps = wo_ps.tile([128, 1024], F32, name="h", tag="h")
                                        for h in range(HPC):
                                            for i in (0, 1):
                                                nc.tensor.matmul(
                                                    ps[:, i * 512:(i + 1) * 512],
                                                    wo_sb[h][:, kc, :],
                                                    oT[h][:, g2 * 1024 + i * 512:
                                                          g2 * 1024 + (i + 1) * 512],
                                                    start=(h == 0), stop=(h == HPC - 1))
                                        st = wo_s.tile([128, 1024], BF16, name="st", tag="st")
                                        nc.vector.tensor_copy(out=st, in_=ps)
                                        for gg in (0, 1):
                                            nc.sync.dma_start(
                                                out=rs_in[s][g2 * 2 + gg,
                                                             kc * 128:(kc + 1) * 128, :],
                                                in_=st[:, gg * 512:(gg + 1) * 512])
                        if sim_local:
                            for g in range(NC):
                                nc.sync.dma_start(out=rs_out[s][:], in_=rs_in[s][g])
                        else:
                            nc.gpsimd.collective_compute(
                                "ReduceScatter", AL.add, replica_groups=rg,
                                ins=[rs_in[s][:]], outs=[rs_out[s][:]])
                qkt_es.close()

                # ---------- phase 5/6: local Wo + FFN + residual + norm per side ----------
                DR = mybir.MatmulPerfMode.DoubleRow
                for s in ("x", "y"):
                    with tc.tile_pool(name="ffn_h", bufs=1) as ffn_h, \
                         tc.tile_pool(name="ffn_g", bufs=1) as ffn_g, \
                         tc.tile_pool(name="ffn_w", bufs=3) as ffn_w, \
                         tc.tile_pool(name="ffn_t", bufs=2) as ffn_t, \
                         tc.tile_pool(name="ffn_ps", bufs=2, space="PSUM") as ffn_ps, \
                         tc.tile_pool(name="ffn_ps1", bufs=1, space="PSUM") as ffn_ps1:
                        hT = []
                        h8 = [ffn_h.tile([128, 2, TPC], FP8, name=f"h8_{i}",
                                         tag=f"h8_{i}") for i in range(KCH // 2)] \
                            if fp8 else None
                        if use_a2a:
                            # gather the 16 heads of my tokens; apply full Wo locally
                            aT = []
                            for hp in range(NC):
                                at = ffn_h.tile([128, 2, TPC], FP8, name=f"aT{hp}",
                                                tag=f"aT{hp}")
                                nc.sync.dma_start(
                                    out=at,
                                    in_=a2a_out[s][hp].rearrange(
                                        "(two p) t -> p two t", p=128))
                                aT.append(at)
                            for kc in range(KCH):
                                wot = ffn_w.tile([128, NC, 2, 128], FP8,
                                                 name="wot", tag="wot")
                                nc.sync.dma_start(out=wot, in_=wo[s][kc])
                                wps = ffn_ps.tile([128, TPC], F32, name="wps", tag="z1")
                                for hp in range(NC):
                                    nc.tensor.matmul(wps, wot[:, hp, :, :], aT[hp],
                                                     start=(hp == 0),
                                                     stop=(hp == NC - 1),
                                                     perf_mode=DR)
                                # h8 = (wps + bo*OSC*WSC) * (FSC/(OSC*WSC))
                                nc.vector.tensor_scalar(
                                    h8[kc // 2][:, kc % 2, :], wps,
                                    bo_sb[s][:, kc:kc + 1], FSC / (OSC * FSC),
                                    op0=AL.add, op1=AL.mult)
                        else:
                            for kc in range(KCH):
                                raw = ffn_t.tile([128, TPC], BF16, name="raw", tag="raw")
                                nc.sync.dma_start(out=raw,
                                                  in_=rs_out[s][kc * 128:(kc + 1) * 128, :])
                                if fp8:
                                    nc.vector.tensor_scalar(
                                        h8[kc // 2][:, kc % 2, :], raw,
                                        bo_sb[s][:, kc:kc + 1], FSC,
                                        op0=AL.add, op1=AL.mult)
                                else:
                                    ht = ffn_h.tile([128, TPC], BF16, name=f"h{kc}",
                                                    tag=f"h{kc}")
                                    nc.vector.tensor_scalar_add(
                                        ht, raw, bo_sb[s][:, kc:kc + 1])
                                    hT.append(ht)
                        g_sb = []
                        for jc in range(JCH):
                            wdt2 = FP8 if fp8 else BF16
                            wsh = [128, KCH // 2, 2, 128] if fp8 else [128, KCH, 128]
                            w1t = ffn_w.tile(wsh, wdt2, name="w1", tag="w1")
                            nc.sync.dma_start(out=w1t, in_=w1[s][jc])
                            w3t = ffn_w.tile(wsh, wdt2, name="w3", tag="w3")
                            nc.sync.dma_start(out=w3t, in_=w3[s][jc])
                            z1 = ffn_ps.tile([128, TPC], F32, name="z1", tag="z1")
                            z3 = ffn_ps.tile([128, TPC], F32, name="z3", tag="z3")
                            if fp8:
                                for kp in range(KCH // 2):
                                    nc.tensor.matmul(z1, w1t[:, kp, :, :], h8[kp],
                                                     start=(kp == 0),
                                                     stop=(kp == KCH // 2 - 1),
                                                     perf_mode=DR)
                                for kp in range(KCH // 2):
                                    nc.tensor.matmul(z3, w3t[:, kp, :, :], h8[kp],
                                                     start=(kp == 0),
                                                     stop=(kp == KCH // 2 - 1),
                                                     perf_mode=DR)
                            else:
                                for kc in range(KCH):
                                    nc.tensor.matmul(z1, w1t[:, kc, :], hT[kc],
                                                     start=(kc == 0), stop=(kc == KCH - 1))
                                for kc in range(KCH):
                                    nc.tensor.matmul(z3, w3t[:, kc, :], hT[kc],
                                                     start=(kc == 0), stop=(kc == KCH - 1))
                            sz = ffn_t.tile([128, TPC], F32, name="sz", tag="sz")
                            nc.scalar.activation(out=sz, in_=z1, func=AF.Silu,
                                                 bias=b1_sb[s][:, jc:jc + 1],
                                                 scale=1.0 / (FSC * FSC) if fp8 else 1.0)
                            gt = ffn_g.tile([128, TPC], BF16, name=f"g{jc}",
                                            tag="gt" if fp8 else f"g{jc}",
                                            bufs=2 if fp8 else None)
                            nc.vector.scalar_tensor_tensor(
                                out=gt, in0=z3, scalar=b3_sb[s][:, jc:jc + 1], in1=sz,
                                op0=AL.add, op1=AL.mult)
                            if fp8:
                                if jc % 2 == 0:
                                    g8 = ffn_g.tile([128, 2, TPC], FP8,
                                                    name=f"g8_{jc // 2}",
                                                    tag=f"g8_{jc // 2}")
                                    g_sb.append(g8)
                                nc.scalar.mul(out=g_sb[jc // 2][:, jc % 2, :],
                                              in_=gt, mul=1.0 / FSC)
                            else:
                                g_sb.append(gt)
                        # W2 pass + residual + stats
                        ffr = []
                        ms_ps = ffn_ps1.tile([1, TPC], F32, name="ms", tag="ms")
                        for kc in range(KCH):
                            wsh2 = [128, JCH // 2, 2, 128] if fp8 else [128, JCH, 128]
                            w2t = ffn_w.tile(wsh2, FP8 if fp8 else BF16,
                                             name="w2", tag="w2", bufs=2)
                            nc.sync.dma_start(out=w2t, in_=w2[s][kc])
                            ff = ffn_ps.tile([128, TPC], F32, name="ff", tag="ff")
                            if fp8:
                                for jp in range(JCH // 2):
                                    nc.tensor.matmul(ff, w2t[:, jp, :, :], g_sb[jp],
                                                     start=(jp == 0),
                                                     stop=(jp == JCH // 2 - 1),
                                                     perf_mode=DR)
                            else:
                                for jc in range(JCH):
                                    nc.tensor.matmul(ff, w2t[:, jc, :], g_sb[jc],
                                                     start=(jc == 0), stop=(jc == JCH - 1))
                            xr = ffn_t.tile([128, TPC], F32, name="xr", tag="xr")
                            nc.sync.dma_start(out=xr, in_=xT[s][kc * 128:(kc + 1) * 128, :])
                            fr = ffn_h.tile([128, TPC], F32, name=f"fr{kc}", tag=f"fr{kc}")
                            if fp8:
                                xr2 = ffn_t.tile([128, TPC], F32, name="xr2", tag="xr2")
                                nc.vector.tensor_scalar_add(
                                    xr2, xr, b2_sb[s][:, kc:kc + 1])
                                nc.vector.scalar_tensor_tensor(
                                    out=fr, in0=ff, scalar=1.0 / (FSC * FSC), in1=xr2,
                                    op0=AL.mult, op1=AL.add)
                            else:
                                nc.vector.scalar_tensor_tensor(
                                    out=fr, in0=ff, scalar=b2_sb[s][:, kc:kc + 1],
                                    in1=xr, op0=AL.add, op1=AL.add)
                            ffr.append(fr)
                            sq = ffn_t.tile([128, TPC], F32, name="fsq", tag="fsq")
                            nc.scalar.activation(out=sq, in_=fr, func=AF.Square)
                            nc.tensor.matmul(ms_ps, ones_f, sq,
                                             start=(kc == 0), stop=(kc == KCH - 1))
                        sd = ffn_t.tile([1, TPC], F32, name="fsd", tag="fsd")
                        nc.scalar.activation(out=sd, in_=ms_ps, func=AF.Sqrt,
                                             bias=eps_sb[0:1, :], scale=1.0 / D)
                        rec = ffn_t.tile([1, TPC], F32, name="frec", tag="frec")
                        nc.vector.reciprocal(out=rec, in_=sd)
                        rb_ps = ffn_ps1.tile([128, TPC], F32, name="frb", tag="frb")
                        nc.tensor.matmul(rb_ps, ones_row, rec, start=True, stop=True)
                        rb = ffn_t.tile([128, TPC], F32, name="frbs", tag="frbs")
                        nc.scalar.copy(out=rb, in_=rb_ps)
                        si = 0 if s == "x" else 1
                        for kc in range(KCH):
                            ot = ffn_t.tile([128, TPC], F32, name="ot", tag="ot")
                            nc.vector.scalar_tensor_tensor(
                                out=ot, in0=ffr[kc], scalar=fnw_sb[s][:, kc:kc + 1],
                                in1=rb, op0=AL.mult, op1=AL.mult)
                            nc.sync.dma_start(
                                out=out_ext[si, kc * 128:(kc + 1) * 128, :], in_=ot)

    nc.compile()
    return nc


def prepare_in_maps(inputs):
    perm = _rope_perm()
    fp8_qkv = USE_FP8 and USE_FP8_QKV
    use_a2a = USE_A2A and USE_FP8
    x = np.asarray(inputs["x"], np.float32).reshape(T, D)
    y = np.asarray(inputs["y"], np.float32).reshape(T, D)
    cos = np.asarray(inputs["freqs_cos"], np.float32).T  # [64, S]
    sin = np.asarray(inputs["freqs_sin"], np.float32).T
    cs = np.concatenate([cos, cos], 0)                   # [128, S]
    sn = np.concatenate([-sin, sin], 0)
    sc = 1.0 / math.sqrt(HD)

    qsc = 1.0 / (FSC * FSC) if fp8_qkv else 1.0
    common = {
        "cs_q": cs * sc * qsc, "sn_q": sn * sc * qsc,
        "cs_k": cs * qsc, "sn_k": sn * qsc,
        "anw": np.asarray(inputs["attn_norm_w"], np.float32).reshape(KCH, 128),
    }

    def tile_lhs(w):  # [K, M] -> [M//128, 128(part=K%), K//128, 128] tiles
        K, M = w.shape
        return np.ascontiguousarray(
            w.reshape(K // 128, 128, M // 128, 128).transpose(2, 1, 0, 3)
        ).astype(BF)

    for s in ("x", "y"):
        if USE_FP8:
            def tile_f8(w):
                K_, M_ = w.shape
                return np.ascontiguousarray(
                    (w * FSC).reshape(K_ // 128, 128, M_ // 128, 128)
                    .transpose(2, 1, 0, 3)).astype(E4)
            common[f"w1_{s}"] = tile_f8(np.asarray(inputs[f"W1_{s}"], np.float32))
            common[f"w3_{s}"] = tile_f8(np.asarray(inputs[f"W3_{s}"], np.float32))
            common[f"w2_{s}"] = tile_f8(np.asarray(inputs[f"W2_{s}"], np.float32))
        else:
            common[f"w1_{s}"] = tile_lhs(np.asarray(inputs[f"W1_{s}"], np.float32))
            common[f"w3_{s}"] = tile_lhs(np.asarray(inputs[f"W3_{s}"], np.float32))
            common[f"w2_{s}"] = tile_lhs(np.asarray(inputs[f"W2_{s}"], np.float32))
        common[f"b1_{s}"] = np.asarray(inputs[f"b1_{s}"], np.float32).reshape(JCH, 128)
        common[f"b3_{s}"] = np.asarray(inputs[f"b3_{s}"], np.float32).reshape(JCH, 128)
        if USE_FP8:
            common[f"b3_{s}"] = common[f"b3_{s}"] * (FSC * FSC)
        common[f"b2_{s}"] = np.asarray(inputs[f"b2_{s}"], np.float32).reshape(KCH, 128)
        bo_np = np.asarray(inputs[f"bo_{s}"], np.float32).reshape(KCH, 128)
        if use_a2a:
            bo_np = bo_np * (OSC * FSC)
        common[f"bo_{s}"] = bo_np
        common[f"fnw_{s}"] = np.asarray(
            inputs[f"ffn_norm_w_{s}"], np.float32).reshape(KCH, 128)
        if use_a2a:
            Wo = np.asarray(inputs[f"Wo_{s}"], np.float32) * FSC
            common[f"wo_{s}"] = np.ascontiguousarray(
                Wo.reshape(H // 2, 2, 128, KCH, 128).transpose(3, 2, 0, 1, 4)
            ).astype(E4)

    in_maps = []
    for c in range(NC):
        m = dict(common)
        m["xT"] = np.ascontiguousarray(x[c * TPC:(c + 1) * TPC].T)
        m["yT"] = np.ascontiguousarray(y[c * TPC:(c + 1) * TPC].T)
        for s in ("x", "y"):
            Wq = np.asarray(inputs[f"Wq_{s}"], np.float32)
            Wk = np.asarray(inputs[f"Wk_{s}"], np.float32)
            Wv = np.asarray(inputs[f"Wv_{s}"], np.float32)
            bqv = np.asarray(inputs[f"bq_{s}"], np.float32)
            bkv = np.asarray(inputs[f"bk_{s}"], np.float32)
            bvv = np.asarray(inputs[f"bv_{s}"], np.float32)
            hsl = [HPC * c + h for h in range(HPC)]
            # [HPC, 128(part=K%), KCH, 128] per-head rope-permuted lhsT tiles
            qw_dt = E4 if fp8_qkv else BF
            qw_sc = FSC if fp8_qkv else 1.0
            def tile_q(w):
                return np.ascontiguousarray(
                    (w * qw_sc).reshape(KCH, 128, 1, 128)
                    .transpose(2, 1, 0, 3))[0].astype(qw_dt)
            wq_t = np.stack([tile_q(Wq[:, h * HD:(h + 1) * HD][:, perm]) for h in hsl])
            wk_t = np.stack([tile_q(Wk[:, h * HD:(h + 1) * HD][:, perm]) for h in hsl])
            m[f"wq_{s}"] = wq_t
            m[f"wk_{s}"] = wk_t
            vcols = np.concatenate([Wv[:, h * HD:(h + 1) * HD] for h in hsl], 1)
            m[f"wv_{s}"] = np.ascontiguousarray(
                (vcols * qw_sc).reshape(KCH, 128, HPC * 128)
                .transpose(1, 0, 2)).astype(qw_dt)
            if not use_a2a:
                Wo = np.asarray(inputs[f"Wo_{s}"], np.float32)
                worows = np.concatenate([Wo[h * HD:(h + 1) * HD, :] for h in hsl], 0)
                m[f"wo_{s}"] = np.ascontiguousarray(
                    worows.reshape(HPC, 128, KCH, 128)).astype(BF)
            bsc = FSC * FSC if fp8_qkv else 1.0
            bq_p = np.stack([bqv[h * HD:(h + 1) * HD][perm] for h in hsl]) * bsc
            bk_p = np.stack([bkv[h * HD:(h + 1) * HD][perm] for h in hsl]) * bsc
            m[f"bq_{s}"] = bq_p
            m[f"bqs_{s}"] = np.concatenate([bq_p[:, 64:], bq_p[:, :64]], 1)
            m[f"bk_{s}"] = bk_p
            m[f"bks_{s}"] = np.concatenate([bk_p[:, 64:], bk_p[:, :64]], 1)
            m[f"bv_{s}"] = np.concatenate(
                [bvv[h * HD:(h + 1) * HD] for h in hsl])
        in_maps.append(m)
    return in_maps


def get_nc():
    if "nc" not in _CACHE:
        _CACHE["nc"] = build_nc()
    return _CACHE["nc"]


def kernel(**inputs):
    nc = get_nc()
    in_maps = prepare_in_maps(inputs)
    res = run_bass_kernel_spmd(nc, in_maps, core_ids=list(range(NC)))
    outs = []
    for si in range(2):
        full = np.concatenate([r["out"][si] for r in res.results], axis=1)  # [D, T]
        outs.append(np.ascontiguousarray(full.T).reshape(B, S, D))
    return outs[0], outs[1]


if __name__ == "__main__":
    nc = get_nc()
    print("build + compile OK")


# revision 21
# speedup vs baseline: 1.0506x; 1.0506x over previous
"""Trainium2 8-core Bass kernel for a dual cross-attention transformer block.

Sharding: tensor-parallel attention heads (16 heads -> 2/core) for both the
x-side and y-side attention; token-parallel FFN (4096 tokens -> 512/core) with
full FFN weights replicated.  Comms: AllGather of the RMS-normed, transposed
activations (fp8) at the start; AllToAll of the per-head attention outputs
(fp8) per side, after which each core applies the FULL Wo to its own 512-token
shard locally (no ReduceScatter).  All activations live feature-on-partition
("transposed") so biases and norm weights are per-partition scalars.  Compute
dtype fp8 (DoubleRow) for all GEMMs except attention scores (bf16); fp32 PSUM.
"""

import math

import numpy as np
import ml_dtypes

import concourse.bass as bass
import concourse.tile as tile
from concourse import mybir, bacc
from concourse.bass_utils import run_bass_kernel_spmd

B, S, D, H = 2, 2048, 2048, 16
HD = D // H            # 128
HID = 5632
EPS = 1e-5
NC = 8                 # cores
HPC = H // NC          # 2 heads per core
T = B * S              # 4096 tokens
TPC = T // NC          # 512 tokens per core
KCH = D // 128         # 16 contraction chunks over D
JCH = HID // 128       # 44 chunks over HID
QBLK = 512             # token block for QKV projections
NQB = T // QBLK        # 8
BF = ml_dtypes.bfloat16
E4 = ml_dtypes.float8_e4m3
F32 = mybir.dt.float32
BF16 = mybir.dt.bfloat16
FP8 = mybir.dt.float8e4
import os as _os
USE_FP8 = _os.environ.get("BASS_FP8", "1") == "1"
USE_FP8_QKV = _os.environ.get("BASS_FP8_QKV", "1") == "1"
USE_A2A = _os.environ.get("BASS_A2A", "1") == "1"
FSC = 16.0
OSC = 16.0             # attention-output fp8 scale (A2A payload)

_CACHE = {}


def _rope_perm():
    # [evens, odds]: puts x1 in partitions 0:64, x2 in 64:128 of Q^T/K^T
    return np.concatenate([np.arange(0, 128, 2), np.arange(1, 128, 2)])


def build_nc(sim_local=False, reps=1, fp8=None):
    if fp8 is None:
        fp8 = USE_FP8
    fp8_qkv = fp8 and USE_FP8_QKV
    use_a2a = USE_A2A and fp8
    nc = bacc.Bacc("TRN2", target_bir_lowering=False, debug=False,
                   num_devices=1 if sim_local else NC)

    AL = mybir.AluOpType
    AF = mybir.ActivationFunctionType

    # ---------------- external parameters ----------------
    xT = {s: nc.declare_dram_parameter(f"{s}T", [D, TPC], F32, isOutput=False)
          for s in ("x", "y")}
    # rope tables, stacked [c;c] and [-s;s]; q-variant pre-scaled by 1/sqrt(HD)
    tabs = {n: nc.declare_dram_parameter(n, [128, S], F32, isOutput=False)
            for n in ("cs_q", "sn_q", "cs_k", "sn_k")}
    wq, wk, wv, wo = {}, {}, {}, {}
    bq, bqs, bk, bks, bv, bo = {}, {}, {}, {}, {}, {}
    w1, w3, w2, b1, b3, b2, fnw = {}, {}, {}, {}, {}, {}, {}
    for s in ("x", "y"):
        qdt = FP8 if fp8_qkv else BF16
        wq[s] = nc.declare_dram_parameter(f"wq_{s}", [HPC, 128, KCH, 128], qdt, isOutput=False)
        wk[s] = nc.declare_dram_parameter(f"wk_{s}", [HPC, 128, KCH, 128], qdt, isOutput=False)
        wv[s] = nc.declare_dram_parameter(f"wv_{s}", [128, KCH, HPC * 128], qdt, isOutput=False)
        if use_a2a:
            # full Wo, fp8, tiled per output chunk for DoubleRow over head pairs
            wo[s] = nc.declare_dram_parameter(f"wo_{s}", [KCH, 128, H // 2, 2, 128],
                                              FP8, isOutput=False)
        else:
            wo[s] = nc.declare_dram_parameter(f"wo_{s}", [HPC, 128, KCH, 128], BF16, isOutput=False)
        bq[s] = nc.declare_dram_parameter(f"bq_{s}", [HPC, 128], F32, isOutput=False)
        bqs[s] = nc.declare_dram_parameter(f"bqs_{s}", [HPC, 128], F32, isOutput=False)
        bk[s] = nc.declare_dram_parameter(f"bk_{s}", [HPC, 128], F32, isOutput=False)
        bks[s] = nc.declare_dram_parameter(f"bks_{s}", [HPC, 128], F32, isOutput=False)
        bv[s] = nc.declare_dram_parameter(f"bv_{s}", [HPC * 128], F32, isOutput=False)
        bo[s] = nc.declare_dram_parameter(f"bo_{s}", [KCH, 128], F32, isOutput=False)
        wdt = FP8 if fp8 else BF16
        if fp8:
            # 4 hidden-chunks per load (one contiguous 2KB run per partition)
            w1[s] = nc.declare_dram_parameter(f"w1_{s}", [JCH // 4, 128, 4, KCH // 2, 2, 128], wdt, isOutput=False)
            w3[s] = nc.declare_dram_parameter(f"w3_{s}", [JCH // 4, 128, 4, KCH // 2, 2, 128], wdt, isOutput=False)
        else:
            w1[s] = nc.declare_dram_parameter(f"w1_{s}", [JCH, 128, KCH, 128], wdt, isOutput=False)
            w3[s] = nc.declare_dram_parameter(f"w3_{s}", [JCH, 128, KCH, 128], wdt, isOutput=False)
        w2[s] = nc.declare_dram_parameter(f"w2_{s}", [KCH, 128, JCH, 128], wdt, isOutput=False)
        b1[s] = nc.declare_dram_parameter(f"b1_{s}", [JCH, 128], F32, isOutput=False)
        b3[s] = nc.declare_dram_parameter(f"b3_{s}", [JCH, 128], F32, isOutput=False)
        b2[s] = nc.declare_dram_parameter(f"b2_{s}", [KCH, 128], F32, isOutput=False)
        fnw[s] = nc.declare_dram_parameter(f"fnw_{s}", [KCH, 128], F32, isOutput=False)
    anw = nc.declare_dram_parameter("anw", [KCH, 128], F32, isOutput=False)
    swp = nc.declare_dram_parameter("swp", [128, 128], BF16, isOutput=False)
    out_ext = nc.declare_dram_parameter("out", [2, D, TPC], F32, isOutput=True)

    # ---------------- internal DRAM ----------------
    # all inter-phase DRAM is pre-tiled partition-major so every DMA is one
    # contiguous run per partition (128 descriptors, cheap HWDGE issue)
    adt = FP8 if fp8_qkv else BF16
    ag_in = {s: nc.dram_tensor(f"ag_in_{s}", [128, KCH, TPC], adt)
             for s in ("x", "y")}
    ag_out = {s: nc.dram_tensor(f"ag_out_{s}", [NC, 128, KCH, TPC], adt,
                                addr_space="Shared")
              for s in ("x", "y")}
    vdt = FP8 if fp8_qkv else BF16
    v_dram = {s: nc.dram_tensor(f"v_{s}", [HPC, B, 128, S // 128, 128], vdt)
              for s in ("x", "y")}
    if use_a2a:
        a2a_in = {s: nc.dram_tensor(f"a2a_in_{s}", [NC, 128, HPC, TPC], FP8)
                  for s in ("x", "y")}
        a2a_out = {s: nc.dram_tensor(f"a2a_out_{s}", [NC, 128, HPC, TPC], FP8)
                   for s in ("x", "y")}
    else:
        rs_in = {s: nc.dram_tensor(f"rs_in_{s}", [NC, D, TPC], BF16) for s in ("x", "y")}
        rs_out = {s: nc.dram_tensor(f"rs_out_{s}", [D, TPC], BF16)
                  for s in ("x", "y")}

    rg = [list(range(NC))]

    from contextlib import ExitStack
    with tile.TileContext(nc) as tc:
        with ExitStack() as es:
            const = es.enter_context(tc.tile_pool(name="const", bufs=1))
            ones_bf = const.tile([128, 1], BF16)
            nc.vector.memset(ones_bf, 1.0)
            ones_f = const.tile([128, 1], F32)
            nc.vector.memset(ones_f, 1.0)
            ones_row = const.tile([1, 128], F32)
            nc.vector.memset(ones_row, 1.0)
            sc_row = const.tile([1, 128], F32)
            nc.vector.memset(sc_row, FSC if fp8_qkv else 1.0)
            eps_sb = const.tile([128, 1], F32)
            nc.vector.memset(eps_sb, EPS)
            anw_sb = const.tile([128, KCH], F32)
            nc.sync.dma_start(out=anw_sb, in_=anw.rearrange("k p -> p k"))
            swp_sb = const.tile([128, 128], BF16)
            nc.sync.dma_start(out=swp_sb, in_=swp[:])
            fnw_sb, bo_sb, b2_sb = {}, {}, {}
            bq_sb, bqs_sb, bk_sb, bks_sb, bv_sb = {}, {}, {}, {}, {}
            b1_sb, b3_sb = {}, {}
            for s in ("x", "y"):
                fnw_sb[s] = const.tile([128, KCH], F32, name=f"fnw{s}", tag=f"fnw{s}")
                nc.sync.dma_start(out=fnw_sb[s], in_=fnw[s].rearrange("k p -> p k"))
                bo_sb[s] = const.tile([128, KCH], F32, name=f"bo{s}", tag=f"bo{s}")
                nc.sync.dma_start(out=bo_sb[s], in_=bo[s].rearrange("k p -> p k"))
                b2_sb[s] = const.tile([128, KCH], F32, name=f"b2{s}", tag=f"b2{s}")
                nc.sync.dma_start(out=b2_sb[s], in_=b2[s].rearrange("k p -> p k"))
                b1_sb[s] = const.tile([128, JCH], F32, name=f"b1{s}", tag=f"b1{s}")
                nc.sync.dma_start(out=b1_sb[s], in_=b1[s].rearrange("k p -> p k"))
                b3_sb[s] = const.tile([128, JCH], F32, name=f"b3{s}", tag=f"b3{s}")
                nc.sync.dma_start(out=b3_sb[s], in_=b3[s].rearrange("k p -> p k"))
                bq_sb[s] = const.tile([128, HPC], F32, name=f"bq{s}", tag=f"bq{s}")
                nc.sync.dma_start(out=bq_sb[s], in_=bq[s].rearrange("h p -> p h"))
                bqs_sb[s] = const.tile([128, HPC], F32, name=f"bqs{s}", tag=f"bqs{s}")
                nc.sync.dma_start(out=bqs_sb[s], in_=bqs[s].rearrange("h p -> p h"))
                bk_sb[s] = const.tile([128, HPC], F32, name=f"bk{s}", tag=f"bk{s}")
                nc.sync.dma_start(out=bk_sb[s], in_=bk[s].rearrange("h p -> p h"))
                bks_sb[s] = const.tile([128, HPC], F32, name=f"bks{s}", tag=f"bks{s}")
                nc.sync.dma_start(out=bks_sb[s], in_=bks[s].rearrange("h p -> p h"))
                bv_sb[s] = const.tile([128, HPC * 128], F32, name=f"bv{s}", tag=f"bv{s}")
                nc.sync.dma_start(out=bv_sb[s],
                                  in_=bv[s][None, :].to_broadcast([128, HPC * 128]))

            for _rep in range(reps):
                # ---------- phase 1: rms-norm own token shard, write ag_in ----------
                with tc.tile_pool(name="nrm", bufs=2) as nrm, \
                     tc.tile_pool(name="nrm_ps", bufs=2, space="PSUM") as nrm_ps:
                    for s in ("x", "y"):
                        xt_sb = []
                        ms_ps = nrm_ps.tile([1, TPC], F32, name="ms", tag="ms")
                        for kc in range(KCH):
                            t = nrm.tile([128, TPC], F32, name="xt", tag="xt", bufs=18)
                            nc.sync.dma_start(out=t, in_=xT[s][kc * 128:(kc + 1) * 128, :])
                            xt_sb.append(t)
                            sq = nrm.tile([128, TPC], F32, name="sq", tag="sq")
                            nc.scalar.activation(out=sq, in_=t, func=AF.Square)
                            nc.tensor.matmul(ms_ps[:, 0:TPC], ones_f, sq,
                                             start=(kc == 0), stop=(kc == KCH - 1))
                        sd = nrm.tile([1, TPC], F32, name="sd", tag="sd")
                        nc.scalar.activation(out=sd, in_=ms_ps, func=AF.Sqrt,
                                             bias=eps_sb[0:1, :], scale=1.0 / D)
                        rec = nrm.tile([1, TPC], F32, name="rec", tag="rec")
                        nc.vector.reciprocal(out=rec, in_=sd)
                        rb_ps = nrm_ps.tile([128, TPC], F32, name="rb", tag="rb")
                        nc.tensor.matmul(rb_ps, sc_row, rec, start=True, stop=True)
                        rb = nrm.tile([128, TPC], F32, name="rbs", tag="rbs")
                        nc.scalar.copy(out=rb, in_=rb_ps)
                        for kc in range(KCH):
                            nt = nrm.tile([128, TPC], FP8 if fp8_qkv else BF16,
                                          name="nt", tag="nt")
                            nc.vector.scalar_tensor_tensor(
                                out=nt, in0=xt_sb[kc], scalar=anw_sb[:, kc:kc + 1],
                                in1=rb, op0=AL.mult, op1=AL.mult)
                            nc.sync.dma_start(out=ag_in[s][:, kc, :], in_=nt)

                # ---------- phase 2: all-gather normalized activations ----------
                for s in ("x", "y"):
                    if sim_local:
                        for g in range(NC):
                            nc.sync.dma_start(out=ag_out[s][g], in_=ag_in[s][:])
                    else:
                        nc.gpsimd.collective_compute(
                            "AllGather", AL.bypass, replica_groups=rg,
                            ins=[ag_in[s][:]], outs=[ag_out[s][:]])

                # ---------- phase 3: QKV projections (both sides) ----------
                qkt_es = ExitStack()
                qkt = qkt_es.enter_context(tc.tile_pool(name="qkt", bufs=1))
                qkdt = FP8 if fp8_qkv else BF16
                qt_sb, kt_sb = {}, {}
                for s in ("x", "y"):
                    for h in range(HPC):
                        qt_sb[(s, h)] = qkt.tile([128, T], qkdt, name=f"qt{s}{h}", tag=f"qt{s}{h}")
                        kt_sb[(s, h)] = qkt.tile([128, T], qkdt, name=f"kt{s}{h}", tag=f"kt{s}{h}")

                with tc.tile_pool(name="qkvw", bufs=1) as qkvw:
                    wq_sb, wk_sb, wv_sb = {}, {}, {}
                    for s in ("x", "y"):
                        for h in range(HPC):
                            wq_sb[(s, h)] = qkvw.tile([128, KCH, 128], FP8 if fp8_qkv else BF16, name=f"wq{s}{h}", tag=f"wq{s}{h}")
                            nc.sync.dma_start(out=wq_sb[(s, h)], in_=wq[s][h])
                            wk_sb[(s, h)] = qkvw.tile([128, KCH, 128], FP8 if fp8_qkv else BF16, name=f"wk{s}{h}", tag=f"wk{s}{h}")
                            nc.sync.dma_start(out=wk_sb[(s, h)], in_=wk[s][h])
                        wv_sb[s] = qkvw.tile([128, KCH, HPC * 128], FP8 if fp8_qkv else BF16, name=f"wv{s}", tag=f"wv{s}")
                        nc.sync.dma_start(out=wv_sb[s], in_=wv[s][:])

                    with tc.tile_pool(name="acts", bufs=2) as acts, \
                         tc.tile_pool(name="ropet", bufs=1) as ropet, \
                         tc.tile_pool(name="ropes", bufs=3) as ropes, \
                         tc.tile_pool(name="qkv_ps", bufs=2, space="PSUM") as qkv_ps:
                        tabs_sb = {}
                        for n in ("cs_q", "sn_q", "cs_k", "sn_k"):
                            tt = ropet.tile([128, S], F32, tag=n)
                            nc.sync.dma_start(out=tt, in_=tabs[n][:])
                            tabs_sb[n] = tt
                        for qb in range(NQB):
                            pos = (qb * QBLK) % S
                            a_sb = {}
                            for s in ("x", "y"):
                                a = acts.tile([128, KCH, QBLK], FP8 if fp8_qkv else BF16, name=f"a{s}", tag=f"a{s}")
                                nc.sync.dma_start(out=a, in_=ag_out[s][qb])
                                a_sb[s] = a
                            tb = {n: tabs_sb[n][:, pos:pos + QBLK]
                                  for n in ("cs_q", "sn_q", "cs_k", "sn_k")}
                            for s in ("x", "y"):
                                src_q = a_sb["y" if s == "x" else "x"]  # queries from hidden
                                src_kv = a_sb[s]                        # keys/values from ctx
                                for h in range(HPC):
                                    for proj, wsb, bsb, bssb, cs_t, sn_t, dst in (
                                        ("q", wq_sb[(s, h)], bq_sb[s], bqs_sb[s],
                                         tb["cs_q"], tb["sn_q"], qt_sb[(s, h)]),
                                        ("k", wk_sb[(s, h)], bk_sb[s], bks_sb[s],
                                         tb["cs_k"], tb["sn_k"], kt_sb[(s, h)]),
                                    ):
                                        src = src_q if proj == "q" else src_kv
                                        ps = qkv_ps.tile([128, QBLK], F32, name="qk", tag="qk")
                                        if fp8_qkv:
                                            for kp in range(KCH // 2):
                                                nc.tensor.matmul(
                                                    ps, wsb[:, 2 * kp:2 * kp + 2, :],
                                                    src[:, 2 * kp:2 * kp + 2, :],
                                                    start=(kp == 0),
                                                    stop=(kp == KCH // 2 - 1),
                                                    perf_mode=mybir.MatmulPerfMode.DoubleRow)
                                        else:
                                            for kc in range(KCH):
                                                nc.tensor.matmul(
                                                    ps, wsb[:, kc, :], src[:, kc, :],
                                                    start=(kc == 0), stop=(kc == KCH - 1))
                                        # rope: copy PSUM->SBUF (bf16), swap
                                        # halves via a permutation matmul on PE,
                                        # then (q+b)*cs + (qsw+bsw)*sn
                                        qs = ropes.tile([128, QBLK], BF16, name="qs", tag="qs")
                                        nc.scalar.copy(out=qs, in_=ps)
                                        qsw = qkv_ps.tile([128, QBLK], F32, name="qsw", tag="qsw")
                                        nc.tensor.matmul(qsw, swp_sb, qs,
                                                         start=True, stop=True)
                                        t1 = ropes.tile([128, QBLK], F32, name="t1", tag="t1")
                                        nc.vector.scalar_tensor_tensor(
                                            out=t1, in0=qs, scalar=bsb[:, h:h + 1],
                                            in1=cs_t, op0=AL.add, op1=AL.mult)
                                        t2 = ropes.tile([128, QBLK], F32, name="t2", tag="t2")
                                        nc.vector.scalar_tensor_tensor(
                                            out=t2, in0=qsw, scalar=bssb[:, h:h + 1],
                                            in1=sn_t, op0=AL.add, op1=AL.mult)
                                        nc.vector.tensor_add(
                                            dst[:, qb * QBLK:(qb + 1) * QBLK], t1, t2)
                                # V in natural [token, hd] layout
                                for tk in range(QBLK // 128):
                                    vps = qkv_ps.tile([128, HPC * 128], F32, name="v", tag="v")
                                    if fp8_qkv:
                                        for kp in range(KCH // 2):
                                            nc.tensor.matmul(
                                                vps,
                                                src_kv[:, 2 * kp:2 * kp + 2,
                                                       tk * 128:(tk + 1) * 128],
                                                wv_sb[s][:, 2 * kp:2 * kp + 2, :],
                                                start=(kp == 0),
                                                stop=(kp == KCH // 2 - 1),
                                                perf_mode=mybir.MatmulPerfMode.DoubleRow)
                                    else:
                                        for kc in range(KCH):
                                            nc.tensor.matmul(
                                                vps, src_kv[:, kc, tk * 128:(tk + 1) * 128],
                                                wv_sb[s][:, kc, :],
                                                start=(kc == 0), stop=(kc == KCH - 1))
                                    vsb = ropes.tile([128, HPC * 128], vdt, name="vsb", tag="vsb")
                                    if fp8_qkv:
                                        # vsb = v*FSC = (vps + bv*FSC^2)/FSC
                                        # (bv host-scaled by FSC; psum FSC^2-scaled)
                                        nc.vector.scalar_tensor_tensor(
                                            out=vsb, in0=vps, scalar=1.0 / FSC,
                                            in1=bv_sb[s], op0=AL.mult, op1=AL.add)
                                    else:
                                        nc.vector.tensor_add(vsb, vps, bv_sb[s])
                                    vb = (qb * QBLK) // S
                                    kcb = (qb % (S // QBLK)) * (QBLK // 128) + tk
                                    for h in (0, 1):
                                        nc.sync.dma_start(
                                            out=v_dram[s][h, vb, :, kcb, :],
                                            in_=vsb[:, h * 128:(h + 1) * 128])

                # ---------- phase 4: attention per side; A2A of head outputs ----------
                for s in ("x", "y"):
                    if use_a2a:
                        with tc.tile_pool(name="att", bufs=2) as att, \
                             tc.tile_pool(name="att_pt", bufs=2) as att_pt, \
                             tc.tile_pool(name="att_ps", bufs=2, space="PSUM") as att_ps, \
                             tc.tile_pool(name="att_ps1", bufs=1, space="PSUM") as att_ps1:
                            for b in range(B):
                                for h in range(HPC):
                                    vsl = att.tile([128, KCH, 128], BF16, name="vsl", tag="vsl")
                                    nc.sync.dma_start(out=vsl, in_=v_dram[s][h, b])
                                    for blk in range(S // 1024):
                                        tq0 = b * S + blk * 1024
                                        pt = []
                                        for tkc in range(16):
                                            sps = att_ps.tile([128, 1024], F32, name="s", tag="s")
                                            for i in (0, 1):
                                                nc.tensor.matmul(
                                                    sps[:, i * 512:(i + 1) * 512],
                                                    kt_sb[(s, h)][:, b * S + tkc * 128:
                                                                  b * S + (tkc + 1) * 128],
                                                    qt_sb[(s, h)][:, tq0 + i * 512:
                                                                  tq0 + (i + 1) * 512],
                                                    start=True, stop=True)
                                            p = att_pt.tile([128, 1024], BF16, name="pt", tag="pt",
                                                            bufs=20)
                                            nc.scalar.activation(out=p, in_=sps, func=AF.Exp)
                                            pt.append(p)
                                        # denominator: DVE pair-tree over tk chunks,
                                        # then one ones-matmul for the partition sum
                                        lvl = pt
                                        li = 0
                                        while len(lvl) > 1:
                                            nxt = []
                                            for i in range(0, len(lvl), 2):
                                                dsum = att_pt.tile(
                                                    [128, 1024], BF16,
                                                    name="dsum", tag=f"ds{li}", bufs=3)
                                                nc.vector.tensor_add(dsum, lvl[i], lvl[i + 1])
                                                nxt.append(dsum)
                                            lvl = nxt
                                            li += 1
                                        ops = att_ps1.tile([128, 1024], F32, name="ops", tag="ops")
                                        den = att_ps1.tile([1, 1024], F32, name="den", tag="aux")
                                        for i in (0, 1):
                                            nc.tensor.matmul(
                                                den[:, i * 512:(i + 1) * 512], ones_bf,
                                                lvl[0][:, i * 512:(i + 1) * 512],
                                                start=True, stop=True)
                                        for tkc in range(16):
                                            for i in (0, 1):
                                                nc.tensor.matmul(
                                                    ops[:, i * 512:(i + 1) * 512],
                                                    vsl[:, tkc, :],
                                                    pt[tkc][:, i * 512:(i + 1) * 512],
                                                    start=(tkc == 0), stop=(tkc == 15))
                                        rec = att.tile([1, 1024], F32, name="rec", tag="rec")
                                        nc.vector.reciprocal(out=rec, in_=den)
                                        rb = att_ps1.tile([128, 1024], F32, name="rb", tag="aux")
                                        for i in (0, 1):
                                            nc.tensor.matmul(rb[:, i * 512:(i + 1) * 512],
                                                             ones_row,
                                                             rec[:, i * 512:(i + 1) * 512],
                                                             start=True, stop=True)
                                        rbs = att.tile([128, 1024], F32, name="rbs",
                                                       tag="rbs")
                                        nc.scalar.copy(out=rbs, in_=rb)
                                        po = att.tile([128, 1024], FP8, name="po",
                                                      tag="po", bufs=3)
                                        nc.vector.scalar_tensor_tensor(
                                            out=po, in0=ops, scalar=OSC,
                                            in1=rbs, op0=AL.mult, op1=AL.mult)
                                        g0 = tq0 // TPC
                                        for gg in (0, 1):
                                            nc.sync.dma_start(
                                                out=a2a_in[s][g0 + gg, :, h, :],
                                                in_=po[:, gg * 512:(gg + 1) * 512])
                        if sim_local:
                            for g in range(NC):
                                nc.sync.dma_start(out=a2a_out[s][g], in_=a2a_in[s][g])
                        else:
                            nc.gpsimd.collective_compute(
                                "AllToAll", AL.bypass, replica_groups=rg,
                                ins=[a2a_in[s][:]], outs=[a2a_out[s][:]])
                    else:
                        oT = {}
                        with tc.tile_pool(name=f"oT{s}", bufs=1) as oT_pool:
                            for h in range(HPC):
                                oT[h] = oT_pool.tile([128, T], BF16, name=f"o{h}", tag=f"o{h}")
                            with tc.tile_pool(name="att", bufs=2) as att, \
                                 tc.tile_pool(name="att_pt", bufs=2) as att_pt, \
                                 tc.tile_pool(name="att_ps", bufs=2, space="PSUM") as att_ps, \
                                 tc.tile_pool(name="att_ps1", bufs=1, space="PSUM") as att_ps1:
                                for b in range(B):
                                    for h in range(HPC):
                                        vsl = att.tile([128, KCH, 128], BF16, name="vsl", tag="vsl")
                                        nc.sync.dma_start(out=vsl, in_=v_dram[s][h, b])
                                        for blk in range(S // 1024):
                                            tq0 = b * S + blk * 1024
                                            pt = []
                                            for tkc in range(16):
                                                sps = att_ps.tile([128, 1024], F32, name="s", tag="s")
                                                for i in (0, 1):
                                                    nc.tensor.matmul(
                                                        sps[:, i * 512:(i + 1) * 512],
                                                        kt_sb[(s, h)][:, b * S + tkc * 128:
                                                                      b * S + (tkc + 1) * 128],
                                                        qt_sb[(s, h)][:, tq0 + i * 512:
                                                                      tq0 + (i + 1) * 512],
                                                        start=True, stop=True)
                                                p = att_pt.tile([128, 1024], BF16, name="pt", tag="pt",
                                                                bufs=20)
                                                nc.scalar.activation(out=p, in_=sps, func=AF.Exp)
                                                pt.append(p)
                                            lvl = pt
                                            li = 0
                                            while len(lvl) > 1:
                                                nxt = []
                                                for i in range(0, len(lvl), 2):
                                                    dsum = att_pt.tile(
                                                        [128, 1024], BF16,
                                                        name="dsum", tag=f"ds{li}", bufs=3)
                                                    nc.vector.tensor_add(dsum, lvl[i], lvl[i + 1])
                                                    nxt.append(dsum)
                                                lvl = nxt
                                                li += 1
                                            ops = att_ps1.tile([128, 1024], F32, name="ops", tag="ops")
                                            den = att_ps1.tile([1, 1024], F32, name="den", tag="aux")
                                            for i in (0, 1):
                                                nc.tensor.matmul(
                                                    den[:, i * 512:(i + 1) * 512], ones_bf,
                                                    lvl[0][:, i * 512:(i + 1) * 512],
                                                    start=True, stop=True)
                                            for tkc in range(16):
                                                for i in (0, 1):
                                                    nc.tensor.matmul(
                                                        ops[:, i * 512:(i + 1) * 512],
                                                        vsl[:, tkc, :],
                                                        pt[tkc][:, i * 512:(i + 1) * 512],
                                                        start=(tkc == 0), stop=(tkc == 15))
                                            rec = att.tile([1, 1024], F32, name="rec", tag="rec")
                                            nc.vector.reciprocal(out=rec, in_=den)
                                            rb = att_ps1.tile([128, 1024], F32, name="rb", tag="aux")
                                            for i in (0, 1):
                                                nc.tensor.matmul(rb[:, i * 512:(i + 1) * 512],
                                                                 ones_row,
                                                                 rec[:, i * 512:(i + 1) * 512],
                                                                 start=True, stop=True)
                                            rbs = att.tile([128, 1024], F32, name="rbs",
                                                           tag="rbs")
                                            nc.scalar.copy(out=rbs, in_=rb)
                                            nc.vector.tensor_mul(
                                                oT[h][:, tq0:tq0 + 1024], ops, rbs)

                            # Wo -> rs_in (token groups are the 512-token core shards)
                            with tc.tile_pool(name="wo", bufs=1) as wo_pool, \
                                 tc.tile_pool(name="wo_s", bufs=3) as wo_s, \
                                 tc.tile_pool(name="wo_ps", bufs=2, space="PSUM") as wo_ps:
                                wo_sb = {}
                                for h in range(HPC):
                                    wo_sb[h] = wo_pool.tile([128, KCH, 128], BF16, name=f"wo{h}", tag=f"wo{h}")
                                    nc.sync.dma_start(out=wo_sb[h], in_=wo[s][h])
                                for g2 in range(NC // 2):
                                    for kc in range(KCH):
                                        ps = wo_ps.tile([128, 1024], F32, name="h", tag="h")
                                        for h in range(HPC):
                                            for i in (0, 1):
                                                nc.tensor.matmul(
                                                    ps[:, i * 512:(i + 1) * 512],
                                                    wo_sb[h][:, kc, :],
                                                    oT[h][:, g2 * 1024 + i * 512:
                                                          g2 * 1024 + (i + 1) * 512],
                                                    start=(h == 0), stop=(h == HPC - 1))
                                        st = wo_s.tile([128, 1024], BF16, name="st", tag="st")
                                        nc.vector.tensor_copy(out=st, in_=ps)
                                        for gg in (0, 1):
                                            nc.sync.dma_start(
                                                out=rs_in[s][g2 * 2 + gg,
                                                             kc * 128:(kc + 1) * 128, :],
                                                in_=st[:, gg * 512:(gg + 1) * 512])
                        if sim_local:
                            for g in range(NC):
                                nc.sync.dma_start(out=rs_out[s][:], in_=rs_in[s][g])
                        else:
                            nc.gpsimd.collective_compute(
                                "ReduceScatter", AL.add, replica_groups=rg,
                                ins=[rs_in[s][:]], outs=[rs_out[s][:]])
                qkt_es.close()

                # ---------- phase 5/6: local Wo + FFN + residual + norm per side ----------
                DR = mybir.MatmulPerfMode.DoubleRow
                for s in ("x", "y"):
                    with tc.tile_pool(name="ffn_h", bufs=1) as ffn_h, \
                         tc.tile_pool(name="ffn_g", bufs=1) as ffn_g, \
                         tc.tile_pool(name="ffn_w", bufs=3) as ffn_w, \
                         tc.tile_pool(name="ffn_t", bufs=2) as ffn_t, \
                         tc.tile_pool(name="ffn_ps", bufs=2, space="PSUM") as ffn_ps, \
                         tc.tile_pool(name="ffn_ps1", bufs=1, space="PSUM") as ffn_ps1:
                        hT = []
                        h8 = [ffn_h.tile([128, 2, TPC], FP8, name=f"h8_{i}",
                                         tag=f"h8_{i}") for i in range(KCH // 2)] \
                            if fp8 else None
                        if use_a2a:
                            # gather the 16 heads of my tokens; apply full Wo locally
                            aT = []
                            for hp in range(NC):
                                at = ffn_h.tile([128, 2, TPC], FP8, name=f"aT{hp}",
                                                tag=f"aT{hp}")
                                nc.sync.dma_start(out=at, in_=a2a_out[s][hp])
                                aT.append(at)
                            for kc in range(KCH):
                                wot = ffn_w.tile([128, NC, 2, 128], FP8,
                                                 name="wot", tag="wot")
                                nc.sync.dma_start(out=wot, in_=wo[s][kc])
                                wps = ffn_ps.tile([128, TPC], F32, name="wps", tag="z1")
                                for hp in range(NC):
                                    nc.tensor.matmul(wps, wot[:, hp, :, :], aT[hp],
                                                     start=(hp == 0),
                                                     stop=(hp == NC - 1),
                                                     perf_mode=DR)
                                # h8 = (wps + bo*OSC*WSC) * (FSC/(OSC*WSC))
                                nc.vector.tensor_scalar(
                                    h8[kc // 2][:, kc % 2, :], wps,
                                    bo_sb[s][:, kc:kc + 1], FSC / (OSC * FSC),
                                    op0=AL.add, op1=AL.mult)
                        else:
                            for kc in range(KCH):
                                raw = ffn_t.tile([128, TPC], BF16, name="raw", tag="raw")
                                nc.sync.dma_start(out=raw,
                                                  in_=rs_out[s][kc * 128:(kc + 1) * 128, :])
                                if fp8:
                                    nc.vector.tensor_scalar(
                                        h8[kc // 2][:, kc % 2, :], raw,
                                        bo_sb[s][:, kc:kc + 1], FSC,
                                        op0=AL.add, op1=AL.mult)
                                else:
                                    ht = ffn_h.tile([128, TPC], BF16, name=f"h{kc}",
                                                    tag=f"h{kc}")
                                    nc.vector.tensor_scalar_add(
                                        ht, raw, bo_sb[s][:, kc:kc + 1])
                                    hT.append(ht)
                        g_sb = []
                        if fp8:
                            for jq in range(JCH // 4):
                                w1t = ffn_w.tile([128, 4, KCH // 2, 2, 128], FP8,
                                                 name="w1", tag="w1")
                                nc.sync.dma_start(out=w1t, in_=w1[s][jq])
                                w3t = ffn_w.tile([128, 4, KCH // 2, 2, 128], FP8,
                                                 name="w3", tag="w3")
                                nc.sync.dma_start(out=w3t, in_=w3[s][jq])
                                for jj in range(4):
                                    jc = jq * 4 + jj
                                    z1 = ffn_ps.tile([128, TPC], F32, name="z1", tag="z1")
                                    z3 = ffn_ps.tile([128, TPC], F32, name="z3", tag="z3")
                                    for kp in range(KCH // 2):
                                        nc.tensor.matmul(z1, w1t[:, jj, kp, :, :], h8[kp],
                                                         start=(kp == 0),
                                                         stop=(kp == KCH // 2 - 1),
                                                         perf_mode=DR)
                                    for kp in range(KCH // 2):
                                        nc.tensor.matmul(z3, w3t[:, jj, kp, :, :], h8[kp],
                                                         start=(kp == 0),
                                                         stop=(kp == KCH // 2 - 1),
                                                         perf_mode=DR)
                                    sz = ffn_t.tile([128, TPC], F32, name="sz", tag="sz")
                                    nc.scalar.activation(out=sz, in_=z1, func=AF.Silu,
                                                         bias=b1_sb[s][:, jc:jc + 1],
                                                         scale=1.0 / (FSC * FSC))
                                    gt = ffn_g.tile([128, TPC], BF16, name=f"g{jc}",
                                                    tag="gt", bufs=2)
                                    nc.vector.scalar_tensor_tensor(
                                        out=gt, in0=z3, scalar=b3_sb[s][:, jc:jc + 1],
                                        in1=sz, op0=AL.add, op1=AL.mult)
                                    if jc % 2 == 0:
                                        g8 = ffn_g.tile([128, 2, TPC], FP8,
                                                        name=f"g8_{jc // 2}",
                                                        tag=f"g8_{jc // 2}")
                                        g_sb.append(g8)
                                    nc.scalar.mul(out=g_sb[jc // 2][:, jc % 2, :],
                                                  in_=gt, mul=1.0 / FSC)
                        else:
                            for jc in range(JCH):
                                w1t = ffn_w.tile([128, KCH, 128], BF16, name="w1", tag="w1")
                                nc.sync.dma_start(out=w1t, in_=w1[s][jc])
                                w3t = ffn_w.tile([128, KCH, 128], BF16, name="w3", tag="w3")
                                nc.sync.dma_start(out=w3t, in_=w3[s][jc])
                                z1 = ffn_ps.tile([128, TPC], F32, name="z1", tag="z1")
                                z3 = ffn_ps.tile([128, TPC], F32, name="z3", tag="z3")
                                for kc in range(KCH):
                                    nc.tensor.matmul(z1, w1t[:, kc, :], hT[kc],
                                                     start=(kc == 0), stop=(kc == KCH - 1))
                                for kc in range(KCH):
                                    nc.tensor.matmul(z3, w3t[:, kc, :], hT[kc],
                                                     start=(kc == 0), stop=(kc == KCH - 1))
                                sz = ffn_t.tile([128, TPC], F32, name="sz", tag="sz")
                                nc.scalar.activation(out=sz, in_=z1, func=AF.Silu,
                                                     bias=b1_sb[s][:, jc:jc + 1])
                                gt = ffn_g.tile([128, TPC], BF16, name=f"g{jc}",
                                                tag=f"g{jc}")
                                nc.vector.scalar_tensor_tensor(
                                    out=gt, in0=z3, scalar=b3_sb[s][:, jc:jc + 1], in1=sz,
                                    op0=AL.add, op1=AL.mult)
                                g_sb.append(gt)
                        # W2 pass + residual + stats
                        ffr = []
                        ms_ps = ffn_ps1.tile([1, TPC], F32, name="ms", tag="ms")
                        for kc in range(KCH):
                            wsh2 = [128, JCH // 2, 2, 128] if fp8 else [128, JCH, 128]
                            w2t = ffn_w.tile(wsh2, FP8 if fp8 else BF16,
                                             name="w2", tag="w2", bufs=2)
                            nc.sync.dma_start(out=w2t, in_=w2[s][kc])
                            ff = ffn_ps.tile([128, TPC], F32, name="ff", tag="ff")
                            if fp8:
                                for jp in range(JCH // 2):
                                    nc.tensor.matmul(ff, w2t[:, jp, :, :], g_sb[jp],
                                                     start=(jp == 0),
                                                     stop=(jp == JCH // 2 - 1),
                                                     perf_mode=DR)
                            else:
                                for jc in range(JCH):
                                    nc.tensor.matmul(ff, w2t[:, jc, :], g_sb[jc],
                                                     start=(jc == 0), stop=(jc == JCH - 1))
                            xr = ffn_t.tile([128, TPC], F32, name="xr", tag="xr")
                            nc.sync.dma_start(out=xr, in_=xT[s][kc * 128:(kc + 1) * 128, :])
                            fr = ffn_h.tile([128, TPC], F32, name=f"fr{kc}", tag=f"fr{kc}")
                            if fp8:
                                xr2 = ffn_t.tile([128, TPC], F32, name="xr2", tag="xr2")
                                nc.vector.tensor_scalar_add(
                                    xr2, xr, b2_sb[s][:, kc:kc + 1])
                                nc.vector.scalar_tensor_tensor(
                                    out=fr, in0=ff, scalar=1.0 / (FSC * FSC), in1=xr2,
                                    op0=AL.mult, op1=AL.add)
                            else:
                                nc.vector.scalar_tensor_tensor(
                                    out=fr, in0=ff, scalar=b2_sb[s][:, kc:kc + 1],
                                    in1=xr, op0=AL.add, op1=AL.add)
                            ffr.append(fr)
                            sq = ffn_t.tile([128, TPC], F32, name="fsq", tag="fsq")
                            nc.scalar.activation(out=sq, in_=fr, func=AF.Square)
                            nc.tensor.matmul(ms_ps, ones_f, sq,
                                             start=(kc == 0), stop=(kc == KCH - 1))
                        sd = ffn_t.tile([1, TPC], F32, name="fsd", tag="fsd")
                        nc.scalar.activation(out=sd, in_=ms_ps, func=AF.Sqrt,
                                             bias=eps_sb[0:1, :], scale=1.0 / D)
                        rec = ffn_t.tile([1, TPC], F32, name="frec", tag="frec")
                        nc.vector.reciprocal(out=rec, in_=sd)
                        rb_ps = ffn_ps1.tile([128, TPC], F32, name="frb", tag="frb")
                        nc.tensor.matmul(rb_ps, ones_row, rec, start=True, stop=True)
                        rb = ffn_t.tile([128, TPC], F32, name="frbs", tag="frbs")
                        nc.scalar.copy(out=rb, in_=rb_ps)
                        si = 0 if s == "x" else 1
                        for kc in range(KCH):
                            ot = ffn_t.tile([128, TPC], F32, name="ot", tag="ot")
                            nc.vector.scalar_tensor_tensor(
                                out=ot, in0=ffr[kc], scalar=fnw_sb[s][:, kc:kc + 1],
                                in1=rb, op0=AL.mult, op1=AL.mult)
                            nc.sync.dma_start(
                                out=out_ext[si, kc * 128:(kc + 1) * 128, :], in_=ot)

    nc.compile()
    return nc


def prepare_in_maps(inputs):
    perm = _rope_perm()
    fp8_qkv = USE_FP8 and USE_FP8_QKV
    use_a2a = USE_A2A and USE_FP8
    x = np.asarray(inputs["x"], np.float32).reshape(T, D)
    y = np.asarray(inputs["y"], np.float32).reshape(T, D)
    cos = np.asarray(inputs["freqs_cos"], np.float32).T  # [64, S]
    sin = np.asarray(inputs["freqs_sin"], np.float32).T
    cs = np.concatenate([cos, cos], 0)                   # [128, S]
    sn = np.concatenate([-sin, sin], 0)
    sc = 1.0 / math.sqrt(HD)

    qsc = 1.0 / (FSC * FSC) if fp8_qkv else 1.0
    # permutation matmul operand: out[m, t] = qs[(m+64)%128, t]
    swp_np = np.zeros((128, 128), np.float32)
    for mm_ in range(128):
        swp_np[(mm_ + 64) % 128, mm_] = 1.0
    common = {
        "cs_q": cs * sc * qsc, "sn_q": sn * sc * qsc,
        "cs_k": cs * qsc, "sn_k": sn * qsc,
        "anw": np.asarray(inputs["attn_norm_w"], np.float32).reshape(KCH, 128),
        "swp": swp_np.astype(BF),
    }

    def tile_lhs(w):  # [K, M] -> [M//128, 128(part=K%), K//128, 128] tiles
        K, M = w.shape
        return np.ascontiguousarray(
            w.reshape(K // 128, 128, M // 128, 128).transpose(2, 1, 0, 3)
        ).astype(BF)

    for s in ("x", "y"):
        if USE_FP8:
            def tile_f8(w):
                K_, M_ = w.shape
                return np.ascontiguousarray(
                    (w * FSC).reshape(K_ // 128, 128, M_ // 128, 128)
                    .transpose(2, 1, 0, 3)).astype(E4)

            def tile_f8_g4(w):
                t = tile_f8(w)                      # [Mc, 128, Kc, 128]
                Mc = t.shape[0]
                return np.ascontiguousarray(
                    t.reshape(Mc // 4, 4, 128, KCH // 2, 2, 128)
                    .transpose(0, 2, 1, 3, 4, 5))
            common[f"w1_{s}"] = tile_f8_g4(np.asarray(inputs[f"W1_{s}"], np.float32))
            common[f"w3_{s}"] = tile_f8_g4(np.asarray(inputs[f"W3_{s}"], np.float32))
            common[f"w2_{s}"] = tile_f8(np.asarray(inputs[f"W2_{s}"], np.float32))
        else:
            common[f"w1_{s}"] = tile_lhs(np.asarray(inputs[f"W1_{s}"], np.float32))
            common[f"w3_{s}"] = tile_lhs(np.asarray(inputs[f"W3_{s}"], np.float32))
            common[f"w2_{s}"] = tile_lhs(np.asarray(inputs[f"W2_{s}"], np.float32))
        common[f"b1_{s}"] = np.asarray(inputs[f"b1_{s}"], np.float32).reshape(JCH, 128)
        common[f"b3_{s}"] = np.asarray(inputs[f"b3_{s}"], np.float32).reshape(JCH, 128)
        if USE_FP8:
            common[f"b3_{s}"] = common[f"b3_{s}"] * (FSC * FSC)
        common[f"b2_{s}"] = np.asarray(inputs[f"b2_{s}"], np.float32).reshape(KCH, 128)
        bo_np = np.asarray(inputs[f"bo_{s}"], np.float32).reshape(KCH, 128)
        if use_a2a:
            bo_np = bo_np * (OSC * FSC)
        common[f"bo_{s}"] = bo_np
        common[f"fnw_{s}"] = np.asarray(
            inputs[f"ffn_norm_w_{s}"], np.float32).reshape(KCH, 128)
        if use_a2a:
            Wo = np.asarray(inputs[f"Wo_{s}"], np.float32) * FSC
            common[f"wo_{s}"] = np.ascontiguousarray(
                Wo.reshape(H // 2, 2, 128, KCH, 128).transpose(3, 2, 0, 1, 4)
            ).astype(E4)

    in_maps = []
    for c in range(NC):
        m = dict(common)
        m["xT"] = np.ascontiguousarray(x[c * TPC:(c + 1) * TPC].T)
        m["yT"] = np.ascontiguousarray(y[c * TPC:(c + 1) * TPC].T)
        for s in ("x", "y"):
            Wq = np.asarray(inputs[f"Wq_{s}"], np.float32)
            Wk = np.asarray(inputs[f"Wk_{s}"], np.float32)
            Wv = np.asarray(inputs[f"Wv_{s}"], np.float32)
            bqv = np.asarray(inputs[f"bq_{s}"], np.float32)
            bkv = np.asarray(inputs[f"bk_{s}"], np.float32)
            bvv = np.asarray(inputs[f"bv_{s}"], np.float32)
            hsl = [HPC * c + h for h in range(HPC)]
            # [HPC, 128(part=K%), KCH, 128] per-head rope-permuted lhsT tiles
            qw_dt = E4 if fp8_qkv else BF
            qw_sc = FSC if fp8_qkv else 1.0
            def tile_q(w):
                return np.ascontiguousarray(
                    (w * qw_sc).reshape(KCH, 128, 1, 128)
                    .transpose(2, 1, 0, 3))[0].astype(qw_dt)
            wq_t = np.stack([tile_q(Wq[:, h * HD:(h + 1) * HD][:, perm]) for h in hsl])
            wk_t = np.stack([tile_q(Wk[:, h * HD:(h + 1) * HD][:, perm]) for h in hsl])
            m[f"wq_{s}"] = wq_t
            m[f"wk_{s}"] = wk_t
            vcols = np.concatenate([Wv[:, h * HD:(h + 1) * HD] for h in hsl], 1)
            m[f"wv_{s}"] = np.ascontiguousarray(
                (vcols * qw_sc).reshape(KCH, 128, HPC * 128)
                .transpose(1, 0, 2)).astype(qw_dt)
            if not use_a2a:
                Wo = np.asarray(inputs[f"Wo_{s}"], np.float32)
                worows = np.concatenate([Wo[h * HD:(h + 1) * HD, :] for h in hsl], 0)
                m[f"wo_{s}"] = np.ascontiguousarray(
                    worows.reshape(HPC, 128, KCH, 128)).astype(BF)
            bsc = FSC * FSC if fp8_qkv else 1.0
            bq_p = np.stack([bqv[h * HD:(h + 1) * HD][perm] for h in hsl]) * bsc
            bk_p = np.stack([bkv[h * HD:(h + 1) * HD][perm] for h in hsl]) * bsc
            m[f"bq_{s}"] = bq_p
            m[f"bqs_{s}"] = np.concatenate([bq_p[:, 64:], bq_p[:, :64]], 1)
            m[f"bk_{s}"] = bk_p
            m[f"bks_{s}"] = np.concatenate([bk_p[:, 64:], bk_p[:, :64]], 1)
            m[f"bv_{s}"] = np.concatenate(
                [bvv[h * HD:(h + 1) * HD] for h in hsl])
        in_maps.append(m)
    return in_maps


def get_nc():
    if "nc" not in _CACHE:
        _CACHE["nc"] = build_nc()
    return _CACHE["nc"]


def kernel(**inputs):
    nc = get_nc()
    in_maps = prepare_in_maps(inputs)
    res = run_bass_kernel_spmd(nc, in_maps, core_ids=list(range(NC)))
    outs = []
    for si in range(2):
        full = np.concatenate([r["out"][si] for r in res.results], axis=1)  # [D, T]
        outs.append(np.ascontiguousarray(full.T).reshape(B, S, D))
    return outs[0], outs[1]


if __name__ == "__main__":
    nc = get_nc()
    print("build + compile OK")
